# revision 18
# baseline (speedup 1.0000x reference)
"""Bass kernel for ClassSeparationLossMargin.

loss = mean_ij [ t*(1-cos) + (1-t)*relu(margin - (1-cos)) ],
cos = xn @ xn.T (row-normalized), t = same-class mask.

Device (per core, on a row-rolled copy so the same SPMD program runs
everywhere): G = H @ H.T with H = [xn | O] (O = one-hot classes, K=1)
=> G = cos + t.  R = weighted sum over scheduled tiles of relu(G + m1).
M = O_local^T @ xn_local ([17, 64] partial class sums over this core's
1024 rows).  Outputs R and M.

Host: A = sum_c n_c^2 (from class_map), M_tot = sum_cores M,
B = sum(M_tot^2), loss = (sum R + dve_offsets - m1*A - 2*B) / N^2.
(Same-class relu passes exactly: 0.1 + cos + 1 >= 0.09 > 0, so the
relu sum is linear there and the correction is exact.)

Triangle: G symmetric; row chunk r processes col chunks c with
(c - r) mod T in {0..T/2}; weight 1 at the span ends, 2 in the middle.
Across the 8 rolled copies every ordered pair is counted exactly once.
Only col tiles 0..39 are ever touched => only those 40 row-tiles are
loaded/normalized/transposed (S needs just the local tiles 0..7).

Transposes go through the DMA xbar ([128,128] bf16 SBUF->SBUF), keeping
ACT/DVE free for the relu+accum consumers (the true bottleneck: PSUM
f32 reads are 1x on both engines).
"""

from contextlib import ExitStack

import numpy as np

import concourse.bacc as bacc
import concourse.mybir as mybir
import concourse.tile as tile
from concourse.masks import make_identity

F32 = mybir.dt.float32
BF16 = mybir.dt.bfloat16
I32 = mybir.dt.int32
OP = mybir.AluOpType
AF = mybir.ActivationFunctionType

P = 128
N = 8192
D = 64
C = 17
MARGIN = 1.1
M1 = MARGIN - 1.0          # 0.1
T = N // P                 # 64 total row tiles
RC = 8                     # row chunks per core
HALF = T // 2              # 32
TU = RC + HALF             # 40: col tiles actually used (0..39)
TG = 8                     # tiles per prep group
NG = TU // TG              # 5 prep groups
HD = D + C                 # 81: hT partition rows (features + one-hot)


def _pack(segs, cap):
    """First-fit-decreasing pack of (r, start_tile, ntiles) segs into
    bins of <= cap tiles; returns list of seg-lists."""
    bins = []
    for seg in sorted(segs, key=lambda s: -s[2]):
        for b in bins:
            if sum(s[2] for s in b) + seg[2] <= cap:
                b.append(seg)
                break
        else:
            bins.append([seg])
    return bins


def build_sched(cap=8):
    """[(segs, weight)] in column-readiness order; segs=(r, tile0, ntiles)."""
    ops = []
    # wave 1: tiles 0..7
    ops.append(([(r, r, 1) for r in range(RC)], 1))
    w2 = [(r, r + 1, 7 - r) for r in range(RC) if 7 - r >= 1]
    ops += [(b, 2) for b in _pack(w2, cap)]
    # waves 2..4: full 8-tile windows
    for k in range(1, 4):
        for r in range(RC):
            ops.append(([(r, 8 * k, 8)], 2))
    # wave 5: tiles 32..39
    w2 = [(r, 32, r) for r in range(RC) if r >= 1]
    ops += [(b, 2) for b in _pack(w2, cap)]
    ops.append(([(r, r + HALF, 1) for r in range(RC)], 1))
    return ops


def _assign_engines(sched):
    """Greedy balance between ACT ('A') and DVE ('D') consumer time."""
    ta = td = 0.0
    out = []
    for (segs, w) in sched:
        fd = 128 * sum(s[2] for s in segs)
        ca = (fd + 172) / 1.2 + 207.0   # ACT: op + accum drain
        cd = (fd + 180) / 0.96          # DVE
        if ta + ca <= td + cd:
            ta += ca
            out.append((segs, w, "A"))
        else:
            td += cd
            out.append((segs, w, "D"))
    return out


def build_nc(n_cores=8, use_xbar=True):
    """Inputs: b_t [128, 40, 64] f32 row-tiled, cm_t [128, 40] i32.
    Outputs: out_r [1,1] f32 relu partial, out_m [17, 64] f32 class sums."""
    nc = bacc.Bacc("TRN2", target_bir_lowering=False, num_devices=n_cores)
    b_dram = nc.dram_tensor("b_t", [P, TU, D], F32, kind="ExternalInput")
    cm_dram = nc.dram_tensor("cm_t", [P, TU], I32, kind="ExternalInput")
    r_dram = nc.dram_tensor("out_r", [1, 1], F32, kind="ExternalOutput")
    m_dram = nc.dram_tensor("out_m", [C, D], F32, kind="ExternalOutput")

    sched = _assign_engines(build_sched())
    n_ops = {("A", 1): 0, ("A", 2): 0, ("D", 1): 0, ("D", 2): 0}
    dve_fdw = 0
    for (segs, w, e) in sched:
        n_ops[(e, w)] += 1
        if e == "D":
            dve_fdw += 128 * sum(s[2] for s in segs) * w
    # DVE accum computes sum max(G, -m1): undercounts m1 per element.
    dve_off = float(M1 * P * dve_fdw)

    with tile.TileContext(nc) as tc, ExitStack() as top:
        persist = top.enter_context(tc.tile_pool(name="persist", bufs=1))
        bpool = top.enter_context(tc.tile_pool(name="bpool", bufs=NG))
        gpool = top.enter_context(tc.tile_pool(name="gpool", bufs=2))
        prep_es = top.enter_context(ExitStack())
        ps_w = prep_es.enter_context(
            tc.tile_pool(name="ps_w", bufs=2, space="PSUM"))
        ps_s = prep_es.enter_context(
            tc.tile_pool(name="ps_s", bufs=1, space="PSUM"))

        # ---- input DMAs first: earliest possible HBM start ----
        b_gs = []
        for g in range(NG):
            b_g = bpool.tile([P, TG, D], F32, tag="b_g", name=f"b_g{g}")
            nc.sync.dma_start(b_g[:], b_dram[:, g * TG:(g + 1) * TG, :])
            b_gs.append(b_g)
        cm_i = persist.tile([P, TU], I32)
        nc.sync.dma_start(cm_i[:], cm_dram[:])

        # ---- constants ----
        bias_m1 = persist.tile([P, 1], F32)
        nc.gpsimd.memset(bias_m1[:], M1)
        # dummy ops: pull the ACT table load (~2.7us) into the DMA window
        warm_act = persist.tile([P, 1], F32)
        nc.scalar.activation(warm_act[:], bias_m1[:], AF.Square)
        nc.scalar.activation(warm_act[:], warm_act[:], AF.Sqrt)
        iota_i = persist.tile([P, TG, C], I32)
        nc.gpsimd.iota(iota_i[:], pattern=[[0, TG], [1, C]], base=0,
                       channel_multiplier=0)
        iotaf = persist.tile([P, TG, C], F32)
        nc.vector.tensor_copy(iotaf[:], iota_i[:])
        ones128 = persist.tile([P, 1], F32)
        nc.gpsimd.memset(ones128[:], 1.0)
        cm_f = persist.tile([P, TU, 1], F32)
        nc.vector.tensor_copy(cm_f[:].squeeze(-1), cm_i[:])
        ident = persist.tile([P, P], BF16)
        make_identity(nc, ident[:])

        # PE warm-up: wide sustained matmuls ramp the PE clock; narrow
        # ones leave it at half rate for the whole main loop.
        for i in range(24):
            wp = ps_w.tile([P, P], F32, tag="warm", name="wp")
            nc.tensor.matmul(wp[:], ident[:], ident[:], start=True, stop=True)

        xno = persist.tile([P, TU, HD], BF16, name="xno")
        GB = TG * P
        hT = [persist.tile([HD, GB], BF16, name=f"hT{g}") for g in range(NG)]
        # lhsT lives in its own tile: LDWEIGHTS (next op) and the rhs
        # stream (current op) would halve PE rate if they shared a tile.
        hL = persist.tile([HD, RC * P], BF16, name="hL")
        s_ps = ps_s.tile([C, D], F32)
        s_sb = persist.tile([C, D], F32)

        hps_pool = cp_i = None
        if not use_xbar:
            hps_pool = prep_es.enter_context(
                tc.tile_pool(name="hps", bufs=2, space="PSUM"))
            cp_i = 0

        # ---- prep: per-group chains; engines balanced so the Tile
        # scheduler pipelines groups (ACT: square/sqrt, DVE: reduce/
        # recip, GPS: normalize-mult + one-hot, DMA: xbar transpose) ----
        for g in range(NG):
            gs = slice(g * TG, (g + 1) * TG)
            sq = gpool.tile([P, TG, D], F32, tag="sq", name=f"sq{g}")
            nc.scalar.activation(sq[:], b_gs[g][:], AF.Square)
            ns = gpool.tile([P, TG], F32, tag="ns", bufs=NG, name=f"ns{g}")
            nc.vector.tensor_reduce(ns[:], sq[:], axis=mybir.AxisListType.X,
                                    op=OP.add)
            nm = gpool.tile([P, TG], F32, tag="nm", bufs=NG, name=f"nm{g}")
            nc.scalar.activation(nm[:], ns[:], AF.Sqrt)
            s_g = gpool.tile([P, TG, 1], F32, tag="s", bufs=NG, name=f"s{g}")
            nc.vector.reciprocal(s_g[:].squeeze(-1), nm[:])
            s_bd = s_g[:].to_broadcast([P, TG, D])
            nc.gpsimd.tensor_tensor(xno[:, gs, 0:D], b_gs[g][:], s_bd,
                                    OP.mult)
            cm_b = cm_f[:, gs, :].to_broadcast([P, TG, C])
            nc.vector.tensor_tensor(xno[:, gs, D:HD], iotaf[:], cm_b,
                                    OP.is_equal)
            if use_xbar:
                for t in range(g * TG, (g + 1) * TG):
                    nc.sync.dma_start_transpose(
                        hT[g][:, (t - g * TG) * P:(t - g * TG + 1) * P],
                        xno[:, t, :])
            else:
                for h in range(2):  # half-groups of 4 tiles
                    hps = hps_pool.tile([HD, 4 * P], F32, tag="hps")
                    for q in range(4):
                        t = g * TG + 4 * h + q
                        nc.tensor.matmul(hps[:, q * P:(q + 1) * P],
                                         xno[:, t, :], ident[:],
                                         start=True, stop=True)
                    base = 4 * h * P
                    cp = (nc.scalar.copy if cp_i % 2 == 0
                          else nc.vector.tensor_copy)
                    cp(hT[g][:, base:base + 4 * P], hps[:])
                    if g == 0:
                        cp2 = (nc.vector.tensor_copy if cp_i % 2 == 0
                               else nc.scalar.copy)
                        cp2(hL[:, base:base + 4 * P], hps[:])
                    cp_i += 1
            if g == 0:
                # local class sums: M = O^T @ xn over this core's rows
                for t in range(RC):
                    nc.tensor.matmul(s_ps[:], xno[:, t, D:D + C],
                                     xno[:, t, 0:D],
                                     start=(t == 0), stop=(t == RC - 1))
                nc.vector.tensor_copy(s_sb[:], s_ps[:])
                nc.sync.dma_start(m_dram[:], s_sb[:])
        # re-warm the PE right before its dense main-loop block
        for i in range(8):
            wp = ps_w.tile([P, P], F32, tag="warm", name="wp")
            nc.tensor.matmul(wp[:], ident[:], ident[:], start=True, stop=True)
        prep_es.close()  # free warm/S/hps PSUM banks for the main ring

        # ---- main loop: G row-blocks -> relu+accum consumers ----
        acc = {}
        for key, cnt in n_ops.items():
            acc[key] = persist.tile([P, max(cnt, 1)], F32,
                                    name=f"acc{key[0]}{key[1]}")
        nxt = {k: 0 for k in acc}
        with tc.tile_pool(name="ps_g", bufs=4, space="PSUM") as ps_g:
            for (segs, w, e) in sched:
                fd = 128 * sum(s[2] for s in segs)
                gt = ps_g.tile([P, fd], F32, name="gt", tag="g")
                x = 0
                for (r, ct, nt) in segs:
                    lhsT = hL[:, r * P:(r + 1) * P]
                    off = ct * P
                    width = nt * P
                    while width > 0:
                        mw = min(512 - (x % 512), width,
                                 GB - (off % GB))
                        nc.tensor.matmul(gt[:, x:x + mw], lhsT,
                                         hT[off // GB][:, off % GB:
                                                       off % GB + mw],
                                         start=True, stop=True)
                        x += mw
                        off += mw
                        width -= mw
                at = acc[(e, w)]
                i = nxt[(e, w)]
                nxt[(e, w)] += 1
                if e == "A":
                    nc.scalar.activation(gt[:], gt[:], AF.Relu,
                                         bias=bias_m1[:, 0:1], scale=1.0,
                                         accum_out=at[:, i:i + 1])
                else:
                    nc.vector.tensor_scalar(gt[:], gt[:], -M1, 0.0,
                                            OP.max, OP.add,
                                            accum_out=at[:, i:i + 1])

            # ---- weighted reduction: R = r1 + 2*r2 ----
            reds = {}
            for key, tl in acc.items():
                rr = persist.tile([P, 1], F32, name=f"red{key[0]}{key[1]}")
                if nxt[key] == 0:
                    nc.gpsimd.memset(rr[:], 0.0)
                else:
                    nc.vector.tensor_reduce(rr[:], tl[:],
                                            axis=mybir.AxisListType.X,
                                            op=OP.add)
                reds[key] = rr
            r1 = persist.tile([P, 1], F32)
            nc.vector.tensor_add(r1[:], reds[("A", 1)][:], reds[("D", 1)][:])
            r2 = persist.tile([P, 1], F32)
            nc.vector.tensor_add(r2[:], reds[("A", 2)][:], reds[("D", 2)][:])
            red = persist.tile([P, 1], F32)
            nc.vector.scalar_tensor_tensor(red[:], r2[:], 2.0, r1[:],
                                           OP.mult, OP.add)
        with tc.tile_pool(name="ps_f", bufs=1, space="PSUM") as ps_f:
            tot_ps = ps_f.tile([1, 1], F32, tag="tot")
            nc.tensor.matmul(tot_ps[:], red[:], ones128[:],
                             start=True, stop=True)
            r_sb = persist.tile([1, 1], F32)
            nc.vector.tensor_copy(r_sb[:], tot_ps[:])
            nc.sync.dma_start(r_dram[:], r_sb[:])

    nc.compile()
    return nc, dict(dve_off=dve_off)


def host_inputs(bottleneck, class_map, n_cores=8):
    """Full inputs -> per-core in_maps (rolled + tiled, first 40 tiles)."""
    roll = N // n_cores
    maps = []
    for c in range(n_cores):
        b = np.roll(bottleneck, -roll * c, axis=0)
        cm = np.roll(class_map, -roll * c, axis=0)
        b_t = np.ascontiguousarray(
            b.reshape(T, P, D).transpose(1, 0, 2)[:, 0:TU, :])
        cm_t = np.ascontiguousarray(cm.reshape(T, P).T[:, 0:TU])
        maps.append({"b_t": b_t.astype(np.float32),
                     "cm_t": cm_t.astype(np.int32)})
    return maps


def host_finalize(results, class_map, dve_off):
    """Combine per-core (out_r, out_m) into the scalar loss."""
    counts = np.bincount(np.asarray(class_map), minlength=C).astype(np.float64)
    A = float((counts ** 2).sum())
    M = np.zeros((C, D), dtype=np.float64)
    R = 0.0
    for res in results:
        R += float(res["out_r"][0, 0]) + dve_off
        M += res["out_m"].astype(np.float64)
    B = float((M ** 2).sum())
    return np.float32((R - M1 * A - 2.0 * B) / (float(N) * N))


# ---------------------------------------------------------------------------
# Harness entry point
# ---------------------------------------------------------------------------
from concourse.bass_utils import run_bass_kernel_spmd

_CACHED = {}


def _get_nc():
    if "nc" not in _CACHED:
        _CACHED["nc"] = build_nc(n_cores=8, use_xbar=False)
    return _CACHED["nc"]


def kernel(bottleneck, class_map):
    bottleneck = np.asarray(bottleneck, dtype=np.float32)
    class_map = np.asarray(class_map, dtype=np.int32)
    nc, meta = _get_nc()
    maps = host_inputs(bottleneck, class_map, n_cores=8)
    res = run_bass_kernel_spmd(nc, maps, core_ids=list(range(8)))
    return host_finalize(res.results, class_map, meta["dve_off"])
